# revision 10
# baseline (speedup 1.0000x reference)
"""CenterLoss update kernel for Trainium2, 8-core SPMD — class-sharded.

Reference computation (N=16384 samples, C=10000 classes, D=128 dims):
    embeded_labels = labels @ center          # [N,D] gather via one-hot
    diff = embeded_labels - embeded_preds
    grad = (labels.T @ diff) / (counts + 1)   # counts = labels.T @ ones
    out  = center - 0.5 * grad

Because each row of ``labels`` is one-hot, ``labels.T @ labels == diag(counts)``,
so the whole thing collapses to a single pass over ``labels``:

    S      = labels.T @ embeded_preds         # [C,D] per-class sum of preds
    counts = column sums of labels            # [C]
    out    = beta * center + gamma * S
             beta  = 1 - 0.5*counts/(counts+1)
             gamma = 0.5/(counts+1)

Sharding: classes (columns of labels) are sharded across the 8 cores.  Each
core streams its own [N, C/8] column block of labels through the PE exactly
once as the moving matmul operand, accumulating S.T = preds.T @ labels in a
single PSUM region over all 128 k-tiles, with per-partition partial counts
accumulated on the vector engine and reduced by PE passes against a ones
vector.  Every core computes its C/8 output shard entirely locally — no
inter-core collective at all.

k-tiles are "virtual": tile q covers sample rows {q + 128*p}.  The
stationary preds tiles are contiguous column slices of preds viewed as
[128, N*D/128] row-major; labels stream two k-tiles per DMA: viewing labels
as [N/2, 2*C/8], pair-tile qq is the strided row slice labels2[qq::64, :],
a contiguous 10 KB DRAM line per descriptor — the measured per-engine sweet
spot (~24.6 GB/s/engine; 4/8/16/20 KB lines and the SWDGE path are all
worse).  Label pairs alternate between the two HWDGE rings (sync/scalar);
preds chunks (10 KB lines too) interleave just ahead of first use; the two
first label DMAs are issued before anything else so the PE starts early.

fp32 matmuls run in the PE's LOW_HIGH mode at roughly the same cols/sec as
bf16 (measured: 213us vs 210us total busy), so no dtype games — f32r end to
end.

Tail structure (everything after the last 10 KB packet is the only part of
the timeline not bound by the stream): counts for pairs < 56 reduce at pair
58, hidden under the stream; pairs 56-62 reduce while the PE waits for the
final pair's DMA; the final pair's counts reduce directly from its label
tile (no DVE add), chunk-by-chunk with the scalar-engine copies trailing
each chunk so the counts->beta/gamma chain pipelines.  beta*center runs
split across the scalar engine and gpsimd in parallel with the PE
transposes of S.T; the output leaves in the same host-permuted
[128, nt3*d] layout center arrives in (5 KB lines instead of a 1250 x 512 B
descriptor storm), un-permuted on the host.
"""

import numpy as np

N, C, D = 16384, 10000, 128
NCORES = 8
CS = C // NCORES   # 1250 classes per core
LR = 0.5
P = 128
KT = N // P        # 128 virtual k-tiles
NPAIR = KT // 2    # 64 pair-tiles
NT3 = (CS + P - 1) // P  # output tiles over the class shard
PCHUNKS = [1024] * 16  # preds chunks (4KB lines, matches baseline exactly)
QQSPLIT = 56            # counts accumulator split point (pair index)
assert sum(PCHUNKS) == KT * D


def _chunks(width, step=512):
    out = []
    c0 = 0
    while c0 < width:
        out.append((c0, min(step, width - c0)))
        c0 += step
    return out


def build_program(cs=CS, d=D, kt=KT):
    """Build the SPMD Bass program (identical on every core)."""
    import concourse.bacc as bacc
    import concourse.mybir as mybir
    import concourse.tile as tile
    from concourse.masks import make_identity

    f32 = mybir.dt.float32
    f32r = mybir.dt.float32r
    mult = mybir.AluOpType.mult
    add = mybir.AluOpType.add

    n = kt * P
    nt3 = NT3
    npair = NPAIR
    assert cs * 4 <= 3 * 2048, "S.T PSUM tile must fit in 3 banks"

    nc = bacc.Bacc(
        "TRN2",
        target_bir_lowering=False,
        debug=False,
        num_devices=NCORES,
    )

    # preds in its natural [128, n] row-major view: partition p holds rows
    # [128p, 128p+128); column block [128q, 128q+128) is then exactly the
    # stationary tile for virtual k-tile q (rows 128p+q on partition p).
    # f32r = raw fp32 bits, so plain HWDGE DMAs feed fp32r matmuls directly.
    preds = nc.dram_tensor("preds", [P, kt * d], f32r, kind="ExternalInput").ap()
    # labels pair view: row r = label rows 2r, 2r+1; pair-tile qq is
    # labels2[qq::64, :] (10 KB contiguous per partition line)
    labels2 = nc.dram_tensor(
        "labels", [n // 2, 2 * cs], f32r, kind="ExternalInput"
    ).ap()
    # center arrives host-permuted: element [p, tt*d + j] = center[tt*P + p, j]
    center = nc.dram_tensor("center", [P, nt3 * d], f32, kind="ExternalInput").ap()
    # out leaves in the same permuted layout; host un-permutes
    out = nc.dram_tensor("out", [P, nt3 * d], f32, kind="ExternalOutput").ap()

    pstart = [sum(PCHUNKS[:c]) for c in range(len(PCHUNKS))]
    # preds chunk c is needed by k-tile pstart[c]/d = pair pstart[c]/(2d);
    # issue a couple of pair-tiles early, alternating between the rings.
    trigger_qq = {}
    for cch in range(len(PCHUNKS)):
        trigger_qq.setdefault(max(0, 4 * cch - 2), []).append(cch)

    with tile.TileContext(nc) as tc:
        with tc.tile_pool(name="const", bufs=1) as const_pool:
            preds_hi = [
                const_pool.tile([P, pw], f32r, name=f"preds_hi_{cch}")
                for cch, pw in enumerate(PCHUNKS)
            ]
            # per-partition partial counts, accumulated on DVE
            acc_a = const_pool.tile([P, 2 * cs], f32, name="acc_a")
            acc_c = const_pool.tile([P, 2 * cs], f32, name="acc_c")
            st_sb = const_pool.tile([d, cs], f32, name="st_sb")
            cnt_row = const_pool.tile([1, cs], f32, name="cnt_row")
            o1_all = const_pool.tile([P, nt3 * d], f32, name="o1_all")
            ou_all = const_pool.tile([P, nt3 * d], f32, name="ou_all")
            identity = const_pool.tile([P, P], f32, name="identity")
            ones_col = const_pool.tile([P, 1], f32, name="ones_col")
            ctr_sb = const_pool.tile([P, nt3 * d], f32, name="ctr_sb")

            # ---------------- phase 1: stream labels ----------------
            with (
                tc.tile_pool(name="lab", bufs=7) as lab_pool,
                tc.tile_pool(name="psum1", bufs=1, space="PSUM") as psum1,
            ):
                st_psum = psum1.tile([d, cs], f32, name="st_psum", space="PSUM")
                cnt_psum = psum1.tile([1, cs], f32, name="cnt_psum", space="PSUM")
                lab_tiles = {}
                for qq in range(npair):
                    # the two first label DMAs go out before anything else
                    lab = lab_pool.tile(
                        [P, 2 * cs], f32r, name=f"lab_{qq}", tag="lab"
                    )
                    lab_tiles[qq] = lab
                    eng = nc.sync if qq % 2 == 0 else nc.scalar
                    eng.dma_start(out=lab[:], in_=labels2[qq::npair, :])
                    if qq == 1:
                        # constants + center while the first tiles are in
                        # flight (center on the otherwise idle gpsimd ring)
                        make_identity(nc, identity[:])
                        nc.vector.memset(ones_col[:], 1.0)
                        nc.vector.memset(ou_all[:], 0.0)
                        nc.gpsimd.dma_start(out=ctr_sb[:], in_=center[:])
                    for cch in trigger_qq.get(qq, []):
                        peng = nc.sync if cch % 2 == 0 else nc.scalar
                        peng.dma_start(
                            out=preds_hi[cch][:],
                            in_=preds[:, pstart[cch]:pstart[cch] + PCHUNKS[cch]],
                        )
                    if qq == npair - 1:
                        # acc_c (pairs 56..62) is final; reducing it here
                        # fills the PE's wait for the final pair's DMA
                        for c0, w in _chunks(cs):
                            for half in (0, 1):
                                nc.tensor.matmul(
                                    out=cnt_psum[0:1, c0:c0 + w],
                                    lhsT=ones_col[:],
                                    rhs=acc_c[:, half * cs + c0:
                                              half * cs + c0 + w],
                                    start=False,
                                    stop=False,
                                )
                    for h in (0, 1):
                        q = 2 * qq + h
                        col = q * d
                        cch = max(
                            i for i in range(len(PCHUNKS)) if pstart[i] <= col
                        )
                        for c0, w in _chunks(cs):
                            nc.tensor.matmul(
                                out=st_psum[:, c0:c0 + w],
                                lhsT=preds_hi[cch][:, col - pstart[cch]:
                                                   col - pstart[cch] + d],
                                rhs=lab[:, h * cs + c0:h * cs + c0 + w],
                                start=(q == 0),
                                stop=(q == kt - 1),
                            )
                    if qq < npair - 1:
                        # counts on DVE; the final pair skips the DVE add and
                        # reduces directly from its tile on the PE below
                        acc = acc_a if qq < QQSPLIT else acc_c
                        if qq in (0, QQSPLIT):
                            nc.vector.tensor_copy(
                                out=acc[:], in_=lab[:].bitcast(f32)
                            )
                        else:
                            nc.vector.tensor_add(
                                out=acc[:], in0=acc[:], in1=lab[:].bitcast(f32)
                            )
                    if qq == QQSPLIT + 2:
                        # acc_a is final; its count reduction (both halves
                        # into the same PSUM regions) hides under the stream
                        for c0, w in _chunks(cs):
                            for half in (0, 1):
                                nc.tensor.matmul(
                                    out=cnt_psum[0:1, c0:c0 + w],
                                    lhsT=ones_col[:],
                                    rhs=acc_a[:, half * cs + c0:
                                              half * cs + c0 + w],
                                    start=(half == 0),
                                    stop=False,
                                )

                # final pair's counts straight from its label tile; close and
                # copy each chunk region immediately so the beta/gamma chain
                # starts while later chunks still reduce
                lab_last = lab_tiles[npair - 1]
                for c0, w in _chunks(cs):
                    for half in (0, 1):
                        nc.tensor.matmul(
                            out=cnt_psum[0:1, c0:c0 + w],
                            lhsT=ones_col[:],
                            rhs=lab_last[:, half * cs + c0:half * cs + c0 + w]
                                .bitcast(f32),
                            start=False,
                            stop=(half == 1),
                        )
                    nc.scalar.copy(
                        out=cnt_row[0:1, c0:c0 + w],
                        in_=cnt_psum[0:1, c0:c0 + w],
                    )
                nc.vector.tensor_copy(out=st_sb[:], in_=st_psum[:])

            # ---------------- phase 3: elementwise update, all local -------
            # counts for all nt3 class tiles land as columns of one [P, nt3]
            # PSUM tile, so beta/gamma come from 5 batched DVE ops; the
            # beta*center products run per-tile on the scalar engine and
            # gpsimd, overlapping the PE transposes of S.T.
            with (
                tc.tile_pool(name="p3", bufs=2) as p3,
                tc.tile_pool(name="psum3", bufs=1, space="PSUM") as psum3,
            ):
                cnt_all = psum3.tile([P, nt3], f32, name="cnt_all", space="PSUM")
                for tt in range(nt3):
                    w = min(P, cs - tt * P)
                    nc.tensor.transpose(
                        out=cnt_all[0:w, tt:tt + 1],
                        in_=cnt_row[0:1, tt * P:tt * P + w],
                        identity=identity[0:1, 0:1],
                    )
                den = p3.tile([P, nt3], f32, name="den", tag="den", bufs=1)
                nc.vector.tensor_scalar_add(out=den[:], in0=cnt_all[:], scalar1=1.0)
                rec = p3.tile([P, nt3], f32, name="rec", tag="rec", bufs=1)
                nc.vector.reciprocal(out=rec[:], in_=den[:])
                gam = p3.tile([P, nt3], f32, name="gam", tag="gam", bufs=1)
                nc.vector.tensor_scalar_mul(out=gam[:], in0=rec[:], scalar1=0.5)
                bet = p3.tile([P, nt3], f32, name="bet", tag="bet", bufs=1)
                nc.vector.tensor_tensor(
                    out=bet[:], in0=cnt_all[:], in1=rec[:], op=mult
                )
                nc.vector.tensor_scalar(
                    out=bet[:], in0=bet[:],
                    scalar1=-0.5, scalar2=1.0, op0=mult, op1=add,
                )

                for tt in range(nt3):
                    w = min(P, cs - tt * P)
                    if tt % 2 == 0:
                        nc.scalar.mul(
                            out=o1_all[0:w, tt * d:tt * d + d],
                            in_=ctr_sb[0:w, tt * d:tt * d + d],
                            mul=bet[0:w, tt:tt + 1],
                        )
                    else:
                        nc.gpsimd.tensor_scalar_mul(
                            out=o1_all[0:w, tt * d:tt * d + d],
                            in0=ctr_sb[0:w, tt * d:tt * d + d],
                            scalar1=bet[0:w, tt:tt + 1],
                        )
                    trp = psum3.tile([P, d], f32, name=f"trp_{tt}", tag="trp",
                                     bufs=4, space="PSUM")
                    nc.tensor.transpose(
                        out=trp[0:w, 0:d],
                        in_=st_sb[:, tt * P:tt * P + w],
                        identity=identity[:, 0:d],
                    )
                    nc.vector.scalar_tensor_tensor(
                        out=ou_all[0:w, tt * d:tt * d + d], in0=trp[0:w, 0:d],
                        scalar=gam[0:w, tt:tt + 1],
                        in1=o1_all[0:w, tt * d:tt * d + d], op0=mult, op1=add,
                    )
                    if tt == nt3 // 2 - 1:
                        # first half of the shard is final: overlap its store
                        nc.sync.dma_start(
                            out=out[:, 0:(nt3 // 2) * d],
                            in_=ou_all[:, 0:(nt3 // 2) * d],
                        )
                nc.scalar.dma_start(
                    out=out[:, (nt3 // 2) * d:nt3 * d],
                    in_=ou_all[:, (nt3 // 2) * d:nt3 * d],
                )

    nc.compile()
    return nc


_PROGRAM = None
LAST_RESULTS = None  # BassKernelResults from the most recent run (for test.py)


def _get_program():
    global _PROGRAM
    if _PROGRAM is None:
        _PROGRAM = build_program()
    return _PROGRAM


def kernel(embeded_preds, labels, center):
    from concourse.bass_utils import run_bass_kernel_spmd

    global LAST_RESULTS
    preds = np.ascontiguousarray(np.asarray(embeded_preds, dtype=np.float32))
    lab = np.ascontiguousarray(np.asarray(labels, dtype=np.float32))
    ctr = np.ascontiguousarray(np.asarray(center, dtype=np.float32))
    assert preds.shape == (N, D) and lab.shape == (N, C) and ctr.shape == (C, D)

    nc = _get_program()
    preds_nat = preds.reshape(P, KT * D)  # free view; bytes unchanged

    def permute_center(cj):
        # [cs, d] -> [P, nt3*d] with [p, tt*d + j] = cj[tt*P + p, j]
        cpad = np.zeros((NT3 * P, D), dtype=np.float32)
        cpad[:cj.shape[0]] = cj
        return np.ascontiguousarray(
            cpad.reshape(NT3, P, D).transpose(1, 0, 2).reshape(P, NT3 * D)
        )

    in_maps = [
        {
            "preds": preds_nat,
            "labels": np.ascontiguousarray(lab[:, j * CS:(j + 1) * CS])
                .reshape(N // 2, 2 * CS),
            "center": permute_center(ctr[j * CS:(j + 1) * CS]),
        }
        for j in range(NCORES)
    ]
    res = run_bass_kernel_spmd(nc, in_maps, core_ids=list(range(NCORES)))
    LAST_RESULTS = res

    def unpermute_out(oj):
        # [P, nt3*d] -> [cs, d]: inverse of permute_center
        return oj.reshape(P, NT3, D).transpose(1, 0, 2).reshape(NT3 * P, D)[:CS]

    return np.concatenate(
        [unpermute_out(res.results[j]["out"]) for j in range(NCORES)], axis=0
    )


# revision 12
# speedup vs baseline: 1.2516x; 1.2516x over previous
"""CenterLoss update kernel for Trainium2, 8-core SPMD — class-sharded (baseline control)."""

import numpy as np

N, C, D = 16384, 10000, 128
NCORES = 8
CS = C // NCORES   # 1250 classes per core
LR = 0.5
P = 128
KT = N // P        # 128 virtual k-tiles
NPAIR = KT // 2    # 64 pair-tiles
NT3 = (CS + P - 1) // P  # output tiles over the class shard
PCHUNKS = [1024] * 16   # preds load chunks (cols of the [128, N] natural view)
QQSPLIT = 56            # counts accumulator split point (pair index)
assert sum(PCHUNKS) == KT * D


def _chunks(width, step=512):
    out = []
    c0 = 0
    while c0 < width:
        out.append((c0, min(step, width - c0)))
        c0 += step
    return out


def build_program(cs=CS, d=D, kt=KT):
    """Build the SPMD Bass program (identical on every core)."""
    import concourse.bacc as bacc
    import concourse.mybir as mybir
    import concourse.tile as tile
    from concourse.masks import make_identity

    f32 = mybir.dt.float32
    f32r = mybir.dt.float32r
    mult = mybir.AluOpType.mult
    add = mybir.AluOpType.add

    n = kt * P
    nt3 = NT3
    npair = NPAIR
    assert cs * 4 <= 3 * 2048, "S.T PSUM tile must fit in 3 banks"

    nc = bacc.Bacc(
        "TRN2",
        target_bir_lowering=False,
        debug=False,
        num_devices=NCORES,
    )

    preds = nc.dram_tensor("preds", [P, kt * d], f32r, kind="ExternalInput").ap()
    labels2 = nc.dram_tensor(
        "labels", [n // 2, 2 * cs], f32r, kind="ExternalInput"
    ).ap()
    center = nc.dram_tensor("center", [P, nt3 * d], f32, kind="ExternalInput").ap()
    out = nc.dram_tensor("out", [cs, d], f32, kind="ExternalOutput").ap()

    trigger_qq = {}
    for cch in range(len(PCHUNKS)):
        trigger_qq.setdefault(max(0, 4 * cch - 2), []).append(cch)

    with tile.TileContext(nc) as tc:
        with tc.tile_pool(name="const", bufs=1) as const_pool:
            identity = const_pool.tile([P, P], f32, name="identity")
            make_identity(nc, identity[:])
            ones_col = const_pool.tile([P, 1], f32, name="ones_col")
            nc.vector.memset(ones_col[:], 1.0)

            ctr_sb = const_pool.tile([P, nt3 * d], f32, name="ctr_sb")
            nc.gpsimd.dma_start(out=ctr_sb[:], in_=center[:])

            preds_hi = [
                const_pool.tile([P, pw], f32r, name=f"preds_hi_{cch}")
                for cch, pw in enumerate(PCHUNKS)
            ]
            pstart = [sum(PCHUNKS[:cch]) for cch in range(len(PCHUNKS))]

            acc_a = const_pool.tile([P, 2 * cs], f32, name="acc_a")
            acc_c = const_pool.tile([P, 2 * cs], f32, name="acc_c")

            st_sb = const_pool.tile([d, cs], f32, name="st_sb")
            cnt_row = const_pool.tile([1, cs], f32, name="cnt_row")

            with (
                tc.tile_pool(name="lab", bufs=7) as lab_pool,
                tc.tile_pool(name="psum1", bufs=1, space="PSUM") as psum1,
            ):
                st_psum = psum1.tile([d, cs], f32, name="st_psum", space="PSUM")
                cnt_psum = psum1.tile([1, cs], f32, name="cnt_psum", space="PSUM")
                for qq in range(npair):
                    for cch in trigger_qq.get(qq, []):
                        peng = nc.sync if cch % 2 == 0 else nc.scalar
                        peng.dma_start(
                            out=preds_hi[cch][:],
                            in_=preds[:, pstart[cch]:pstart[cch] + PCHUNKS[cch]],
                        )
                    lab2 = lab_pool.tile(
                        [P, 2 * cs], f32r, name=f"lab_{qq}", tag="lab"
                    )
                    eng = nc.sync if qq % 2 == 0 else nc.scalar
                    eng.dma_start(out=lab2[:], in_=labels2[qq::npair, :])
                    for h in (0, 1):
                        q = 2 * qq + h
                        col = q * d
                        cch = max(
                            i for i in range(len(PCHUNKS)) if pstart[i] <= col
                        )
                        for c0, w in _chunks(cs):
                            nc.tensor.matmul(
                                out=st_psum[:, c0:c0 + w],
                                lhsT=preds_hi[cch][:, col - pstart[cch]:
                                                   col - pstart[cch] + d],
                                rhs=lab2[:, h * cs + c0:h * cs + c0 + w],
                                start=(q == 0),
                                stop=(q == kt - 1),
                            )
                    acc = acc_a if qq < QQSPLIT else acc_c
                    if qq in (0, QQSPLIT):
                        nc.vector.tensor_copy(out=acc[:], in_=lab2[:].bitcast(f32))
                    else:
                        nc.vector.tensor_add(
                            out=acc[:], in0=acc[:], in1=lab2[:].bitcast(f32)
                        )
                    if qq == QQSPLIT + 2:
                        for h in (0, 1):
                            for c0, w in _chunks(cs):
                                nc.tensor.matmul(
                                    out=cnt_psum[0:1, c0:c0 + w],
                                    lhsT=ones_col[:],
                                    rhs=acc_a[:, h * cs + c0:h * cs + c0 + w],
                                    start=(h == 0),
                                    stop=False,
                                )

                for h in (0, 1):
                    for c0, w in _chunks(cs):
                        nc.tensor.matmul(
                            out=cnt_psum[0:1, c0:c0 + w],
                            lhsT=ones_col[:],
                            rhs=acc_c[:, h * cs + c0:h * cs + c0 + w],
                            start=False,
                            stop=(h == 1),
                        )
                nc.scalar.copy(out=cnt_row[:], in_=cnt_psum[:])
                nc.scalar.copy(out=st_sb[:], in_=st_psum[:])

            with (
                tc.tile_pool(name="p3", bufs=2) as p3,
                tc.tile_pool(name="psum3", bufs=1, space="PSUM") as psum3,
            ):
                cnt_all = psum3.tile([P, nt3], f32, name="cnt_all", space="PSUM")
                for tt in range(nt3):
                    w = min(P, cs - tt * P)
                    nc.tensor.transpose(
                        out=cnt_all[0:w, tt:tt + 1],
                        in_=cnt_row[0:1, tt * P:tt * P + w],
                        identity=identity[0:1, 0:1],
                    )
                den = p3.tile([P, nt3], f32, name="den", tag="den", bufs=1)
                nc.vector.tensor_scalar_add(out=den[:], in0=cnt_all[:], scalar1=1.0)
                rec = p3.tile([P, nt3], f32, name="rec", tag="rec", bufs=1)
                nc.vector.reciprocal(out=rec[:], in_=den[:])
                gam = p3.tile([P, nt3], f32, name="gam", tag="gam", bufs=1)
                nc.vector.tensor_scalar_mul(out=gam[:], in0=rec[:], scalar1=0.5)
                bet = p3.tile([P, nt3], f32, name="bet", tag="bet", bufs=1)
                nc.vector.tensor_tensor(
                    out=bet[:], in0=cnt_all[:], in1=rec[:], op=mult
                )
                nc.vector.tensor_scalar(
                    out=bet[:], in0=bet[:],
                    scalar1=-0.5, scalar2=1.0, op0=mult, op1=add,
                )

                o1_all = p3.tile([P, nt3 * d], f32, name="o1_all", tag="o1",
                                 bufs=1)
                nc.vector.tensor_tensor(
                    out=o1_all[:].rearrange("p (t x) -> p t x", x=d),
                    in0=ctr_sb[:].rearrange("p (t x) -> p t x", x=d),
                    in1=bet[:].unsqueeze(2).broadcast_to([P, nt3, d]),
                    op=mult,
                )

                ou_all = p3.tile([P, nt3 * d], f32, name="ou_all", tag="ou",
                                 bufs=1)
                for tt in range(nt3):
                    w = min(P, cs - tt * P)
                    trp = psum3.tile([P, d], f32, name=f"trp_{tt}", tag="trp",
                                     bufs=4, space="PSUM")
                    nc.tensor.transpose(
                        out=trp[0:w, 0:d],
                        in_=st_sb[:, tt * P:tt * P + w],
                        identity=identity[:, 0:d],
                    )
                    nc.vector.scalar_tensor_tensor(
                        out=ou_all[0:w, tt * d:tt * d + d], in0=trp[0:w, 0:d],
                        scalar=gam[0:w, tt:tt + 1],
                        in1=o1_all[0:w, tt * d:tt * d + d], op0=mult, op1=add,
                    )
                nc.sync.dma_start(
                    out=out[0:(nt3 - 1) * P, :]
                        .rearrange("(t p) x -> p t x", p=P),
                    in_=ou_all[:, 0:(nt3 - 1) * d]
                        .rearrange("p (t x) -> p t x", x=d),
                )
                wlast = cs - (nt3 - 1) * P
                nc.scalar.dma_start(
                    out=out[(nt3 - 1) * P:cs, :],
                    in_=ou_all[0:wlast, (nt3 - 1) * d:nt3 * d],
                )

    nc.compile()
    return nc


_PROGRAM = None
LAST_RESULTS = None


def _get_program():
    global _PROGRAM
    if _PROGRAM is None:
        _PROGRAM = build_program()
    return _PROGRAM


def kernel(embeded_preds, labels, center):
    from concourse.bass_utils import run_bass_kernel_spmd

    global LAST_RESULTS
    preds = np.ascontiguousarray(np.asarray(embeded_preds, dtype=np.float32))
    lab = np.ascontiguousarray(np.asarray(labels, dtype=np.float32))
    ctr = np.ascontiguousarray(np.asarray(center, dtype=np.float32))
    assert preds.shape == (N, D) and lab.shape == (N, C) and ctr.shape == (C, D)

    nc = _get_program()
    preds_nat = preds.reshape(P, KT * D)

    def permute_center(cj):
        cpad = np.zeros((NT3 * P, D), dtype=np.float32)
        cpad[:cj.shape[0]] = cj
        return np.ascontiguousarray(
            cpad.reshape(NT3, P, D).transpose(1, 0, 2).reshape(P, NT3 * D)
        )

    in_maps = [
        {
            "preds": preds_nat,
            "labels": np.ascontiguousarray(lab[:, j * CS:(j + 1) * CS])
                .reshape(N // 2, 2 * CS),
            "center": permute_center(ctr[j * CS:(j + 1) * CS]),
        }
        for j in range(NCORES)
    ]
    res = run_bass_kernel_spmd(nc, in_maps, core_ids=list(range(NCORES)))
    LAST_RESULTS = res
    return np.concatenate([res.results[j]["out"] for j in range(NCORES)], axis=0)
